# revision 1
# baseline (speedup 1.0000x reference)
"""Delay-and-sum beamformer on 8 TRN2 NeuronCores.

Problem: x[16, 100000, 128] f32 -> out[b, t] = mean_s x[b, t + d_s, s],
d_s = round(s * sin(30deg) / 2) in [0, 32] (zero-padded past t = T-1).

Sharding: pure data parallel over batch (2 batches per core).

Per-core layout ("stripe" scheme): for each batch, partition p owns time
rows [784*p, 784*(p+1)) of a zero-padded T_pad = 100384 signal.  Each
partition's rows live contiguously in its SBUF free dim as [row, sensor]
(row pitch 128 f32), loaded in 7 chunks of 112 rows through a 3-slot ring
whose third slot is 32 rows "fat" (its compute window is self-contained).
Chunks are processed in descending time order so a chunk's +32-row halo is
always already resident; fat-slot halos are SBUF-copied from the resident
neighbor instead of re-read, so HBM traffic is ~104% of the input size.

The delayed sensor sum exploits the delay structure: d groups sensors as
{0,1,2} (d=0), {4k-1..4k+2} (d=k, k=1..31), {127} (d=32).  In the
flattened per-partition layout the taps for output row tau sit at
  tau*128 + {0,1,2},  tau*128 + 131 + 132*k + j (k=0..30, j=0..3),
  tau*128 + 4223,
so the whole 128-sensor shifted sum is two strided tensor_reduce ops +
two adds + one scale on the vector engine (~1 input elem/cycle).
"""

import numpy as np

B, T, S = 16, 100000, 128
NCORES = 8
BC = B // NCORES          # batches per core
LS = 784                  # stripe rows per partition (128*784 = 100352 >= T)
TP = 128 * LS             # padded output rows per batch
HALO = 32                 # max delay
TPAD = TP + HALO          # padded input rows per batch
LC = 112                  # chunk rows
NCH = LS // LC            # 7 chunks per batch
RING_ROWS = 3 * LC + HALO  # 3 ring slots + shadow
SCALE = 1.0 / S

_cache = {}


def _build():
    import concourse.bass as bass
    import concourse.tile as tile
    from concourse import bacc, mybir

    f32 = mybir.dt.float32
    nc = bacc.Bacc("TRN2", target_bir_lowering=False, debug=False, num_devices=1)
    x = nc.dram_tensor("x", [BC * TPAD * S], f32, kind="ExternalInput")
    y = nc.dram_tensor("y", [BC * TP], f32, kind="ExternalOutput")

    def dram_ap(base_elem, rows):
        # [128 partitions (stripe-major), rows*S contiguous elems each]
        return bass.AP(x.ap().tensor, base_elem, [[LS * S, 128], [1, rows * S]])

    def sub_ap(t, off, dims):
        # custom AP into a tile: keep its partition dim, replace free dims
        return bass.AP(t.tensor, t.offset + off, [list(t.ap[0])] + dims)

    with tile.TileContext(nc) as tc:
        from contextlib import ExitStack

        with ExitStack() as ctx:
            ring_pool = ctx.enter_context(tc.tile_pool(name="ring", bufs=1))
            g_pool = ctx.enter_context(tc.tile_pool(name="g", bufs=2))
            o_pool = ctx.enter_context(tc.tile_pool(name="o", bufs=1))

            ring = ring_pool.tile([128, RING_ROWS * S], f32)

            def compute(c, slot, out_sb, t0=0, t1=LC):
                # produce out rows [c*LC+t0, c*LC+t1) from ring rows
                # [slot*LC+t0, slot*LC+t1+32)
                n = t1 - t0
                base = slot * LC * S + t0 * S
                g1 = g_pool.tile([128, n], f32, tag="g1")
                g2 = g_pool.tile([128, n], f32, tag="g2")
                # groups k=1..31 (sensors 3..126): rows tau+1..tau+31
                nc.vector.reduce_sum(
                    g1[:],
                    sub_ap(ring, base + 131, [[S, n], [132, 31], [1, 4]]),
                    axis=mybir.AxisListType.XY,
                )
                # group d=0 (sensors 0..2) at row tau
                nc.vector.reduce_sum(
                    g2[:],
                    sub_ap(ring, base, [[S, n], [1, 3]]),
                    axis=mybir.AxisListType.X,
                )
                nc.vector.tensor_add(g1[:], g1[:], g2[:])
                # sensor 127 (d=32) at row tau+32, col 127
                nc.vector.tensor_add(
                    g1[:], g1[:], sub_ap(ring, base + 4223, [[S, n]])
                )
                nc.vector.tensor_scalar_mul(
                    out_sb[:, c * LC + t0 : c * LC + t1], g1[:], SCALE
                )

            # Chunks are processed in DESCENDING order: compute c reads its
            # own chunk plus the first 32 rows of chunk c+1, which is then
            # always already resident -- computes never wait on a future
            # load and loads never WAR-stall on pending computes.  Slot map
            # slot(c) = (c + off) % 3 keeps compute windows contiguous
            # (chunk c+1 sits physically after chunk c in the ring).  Slot 2
            # is "fat": its load covers LC+32 rows so its compute is
            # self-contained (ring rows [224, 368)).
            #   b0: off=2 -> fats at c6,c3,c0; the fat c6 load covers the
            #       stripe tail (host-padded), so no tail DMA.
            #   b1: off=0 -> first load (chunk 6, slot 0) only WARs b0's
            #       second-to-last compute, so the batch handoff does not
            #       stall; chunk 6's compute needs the stripe tail in slot 1
            #       rows [0,32), loaded first.
            offsets = [2, 0]

            for b in range(BC):
                xb = b * TPAD * S
                off = offsets[b]
                out_sb = o_pool.tile([128, LS], f32, tag="out_sb")
                if (NCH - 1 + off) % 3 != 2:
                    # stripe-tail halo for chunk NCH-1's compute: rows
                    # [LS, LS+32) -> the head of the slot after its own.
                    tslot = (NCH + off) % 3
                    nc.sync.dma_start(
                        sub_ap(ring, tslot * LC * S, [[1, HALO * S]]),
                        dram_ap(xb + LS * S, HALO),
                    )
                for c in range(NCH - 1, -1, -1):
                    slot = (c + off) % 3
                    if b == BC - 1 and c == 0:
                        # last chunk of the kernel: split load+compute to
                        # shorten the end-of-kernel drain.
                        mid = 56
                        nc.sync.dma_start(
                            sub_ap(ring, (slot * LC + mid) * S, [[1, (LC - mid) * S]]),
                            dram_ap(xb + (c * LC + mid) * S, LC - mid),
                        )
                        compute(c, slot, out_sb, mid, LC)
                        nc.sync.dma_start(
                            sub_ap(ring, slot * LC * S, [[1, mid * S]]),
                            dram_ap(xb + c * LC * S, mid),
                        )
                        compute(c, slot, out_sb, 0, mid)
                        continue
                    if slot == 2 and c < NCH - 1:
                        # fat slot, but its last 32 rows (= chunk c+1 rows
                        # [0,32)) are already resident at slot 0: load only
                        # LC rows from HBM and SBUF-copy the halo instead.
                        nc.sync.dma_start(
                            sub_ap(ring, 2 * LC * S, [[1, LC * S]]),
                            dram_ap(xb + c * LC * S, LC),
                        )
                        nc.sync.dma_start(
                            sub_ap(ring, 3 * LC * S, [[1, HALO * S]]),
                            sub_ap(ring, 0, [[1, HALO * S]]),
                        )
                    else:
                        rows = LC + HALO if slot == 2 else LC
                        nc.sync.dma_start(
                            sub_ap(ring, slot * LC * S, [[1, rows * S]]),
                            dram_ap(xb + c * LC * S, rows),
                        )
                    compute(c, slot, out_sb)

                nc.sync.dma_start(
                    bass.AP(y.ap().tensor, b * TP, [[LS, 128], [1, LS]]),
                    out_sb[:],
                )

    nc.compile()
    return nc


def _get_nc():
    if "nc" not in _cache:
        _cache["nc"] = _build()
    return _cache["nc"]


def kernel(microphone_array: np.ndarray) -> np.ndarray:
    from concourse.bass_utils import run_bass_kernel_spmd

    x = np.asarray(microphone_array, dtype=np.float32)
    assert x.shape == (B, T, S)
    nc = _get_nc()

    in_maps = []
    for c in range(NCORES):
        shard = np.zeros((BC, TPAD, S), dtype=np.float32)
        shard[:, :T] = x[c * BC : (c + 1) * BC]
        in_maps.append({"x": shard.reshape(-1)})

    res = _cache["res"] = run_bass_kernel_spmd(
        nc, in_maps, core_ids=list(range(NCORES)), trace=_cache.get("trace", False)
    )

    out = np.empty((B, T), dtype=np.float32)
    for c in range(NCORES):
        out[c * BC : (c + 1) * BC] = res.results[c]["y"].reshape(BC, TP)[:, :T]
    return out



# revision 4
# speedup vs baseline: 1.3524x; 1.3524x over previous
"""Delay-and-sum beamformer on 8 TRN2 NeuronCores (bf16 pipeline).

Problem: x[16, 100000, 128] f32 -> out[b, t] = mean_s x[b, t + d_s, s],
d_s = round(s * sin(30deg) / 2) in [0, 32] (zero-padded past t = T-1).

Sharding: pure data parallel over batch (2 batches per core).

The rel-err gate is 2e-2; bf16 round-to-nearest costs ~4e-3, so the host
shard step converts to bf16 (folding the 1/128 mean scale into the
conversion -- a power of two, so it is exact) and the device pipeline is
bf16 end to end.  This halves DMA bytes and enables the DVE 2x perf mode
(2-byte dtype, packed inner dim) for the big strided reduce.

Per-core layout ("stripe" scheme): for each batch, partition p owns time
rows [784*p, 784*(p+1)) of a zero-padded T_pad = 100384 signal, rows
contiguous in the SBUF free dim as [row, sensor] (row pitch 128 bf16 =
256 B).  Rows stream through a 6-slot ring of 98-row chunks (8 chunks
per batch) in DESCENDING time order, so a chunk's +32-row halo is the
already-resident next chunk at the adjacent slot.  The ring has one
32-row shadow after slot 5; chunks landing on slot 5 are loaded "fat"
(130 rows) so their compute window is self-contained.  With 8 chunks
over 6 slots the first/last chunks of a batch never collide on a slot,
so batch handoff does not stall the queue.

The delayed sensor sum exploits the delay structure: d groups sensors as
{0,1,2} (d=0), {4k-1..4k+2} (d=k, k=1..31), {127} (d=32).  Per chunk:
  DVE:    g1[tau] = strided reduce of the 31 4-wide groups (2x mode)
  GPSIMD: g2[tau] = reduce of sensors {0,1,2} at row tau,
          g2 += x[tau+32, 127]
  DVE:    out[tau] = g1 + g2   (input was pre-scaled by 1/128)
Loads run on the SP HWDGE queue; output stores run on the Activation
HWDGE queue so they never head-of-line-block the loads.
"""

import numpy as np

B, T, S = 16, 100000, 128
NCORES = 8
BC = B // NCORES          # batches per core
LS = 784                  # stripe rows per partition (128*784 = 100352 >= T)
TP = 128 * LS             # padded output rows per batch
HALO = 32                 # max delay
TPAD = TP + HALO          # padded input rows per batch
LC = 98                   # chunk rows
NCH = LS // LC            # 8 chunks per batch
NSLOT = 6                 # ring slots
OFF = 4                   # slot(c) = (c + OFF) % NSLOT; c=7 -> 5 (shadowed)
RING_ROWS = NSLOT * LC + HALO   # 620 rows * 256 B = 158.7 KiB / partition

_cache = {}


def _build():
    import concourse.bass as bass
    import concourse.tile as tile
    from concourse import bacc, mybir

    bf16 = mybir.dt.bfloat16
    nc = bacc.Bacc("TRN2", target_bir_lowering=False, debug=False, num_devices=1)
    x = nc.dram_tensor("x", [BC * TPAD * S], bf16, kind="ExternalInput")
    y = nc.dram_tensor("y", [BC * TP], bf16, kind="ExternalOutput")

    def dram_ap(base_elem, rows):
        # [128 partitions (stripe-major), rows*S contiguous elems each]
        return bass.AP(x.ap().tensor, base_elem, [[LS * S, 128], [1, rows * S]])

    def sub_ap(t, off, dims):
        # custom AP into a tile: keep its partition dim, replace free dims
        return bass.AP(t.tensor, t.offset + off, [list(t.ap[0])] + dims)

    with tile.TileContext(nc) as tc:
        from contextlib import ExitStack

        with ExitStack() as ctx:
            ctx.enter_context(
                nc.allow_low_precision(
                    reason="bf16 sums; rel-err gate is 2e-2, bf16 costs ~4e-3"
                )
            )
            ring_pool = ctx.enter_context(tc.tile_pool(name="ring", bufs=1))
            g_pool = ctx.enter_context(tc.tile_pool(name="g", bufs=2))
            o_pool = ctx.enter_context(tc.tile_pool(name="o", bufs=2))

            ring = ring_pool.tile([128, RING_ROWS * S], bf16)

            def compute(c, slot, out_sb, t0=0, t1=LC):
                # produce out rows [c*LC+t0, c*LC+t1) from ring rows
                # [slot*LC+t0, slot*LC+t1+32)
                n = t1 - t0
                base = slot * LC * S + t0 * S
                g1 = g_pool.tile([128, n], bf16, tag="g1")
                g2 = g_pool.tile([128, n], bf16, tag="g2")
                # groups d=1..31 (sensors 3..126): rows tau+1..tau+31
                nc.vector.reduce_sum(
                    g1[:],
                    sub_ap(ring, base + 131, [[S, n], [132, 31], [1, 4]]),
                    axis=mybir.AxisListType.XY,
                )
                # group d=0 (sensors 0..2) at row tau + sensor 127 (d=32) at
                # row tau+32 -- built from adds on GPSIMD (its tensor_reduce
                # only does cross-partition axes)
                nc.gpsimd.tensor_add(
                    g2[:],
                    sub_ap(ring, base, [[S, n]]),
                    sub_ap(ring, base + 1, [[S, n]]),
                )
                nc.gpsimd.tensor_add(
                    g2[:], g2[:], sub_ap(ring, base + 2, [[S, n]])
                )
                nc.gpsimd.tensor_add(
                    g2[:], g2[:], sub_ap(ring, base + HALO * S + 127, [[S, n]])
                )
                nc.vector.tensor_add(
                    out_sb[:, c * LC + t0 : c * LC + t1], g1[:], g2[:]
                )

            for b in range(BC):
                xb = b * TPAD * S
                out_sb = o_pool.tile([128, LS], bf16, tag="out_sb")
                for c in range(NCH - 1, -1, -1):
                    slot = (c + OFF) % NSLOT
                    rows = LC + HALO if slot == NSLOT - 1 else LC
                    if b == BC - 1 and c == 0:
                        # last chunk of the kernel: split load+compute to
                        # shorten the end-of-kernel drain.
                        mid = LC // 2
                        nc.sync.dma_start(
                            sub_ap(ring, (slot * LC + mid) * S, [[1, (LC - mid) * S]]),
                            dram_ap(xb + (c * LC + mid) * S, LC - mid),
                        )
                        compute(c, slot, out_sb, mid, LC)
                        nc.sync.dma_start(
                            sub_ap(ring, slot * LC * S, [[1, mid * S]]),
                            dram_ap(xb + c * LC * S, mid),
                        )
                        compute(c, slot, out_sb, 0, mid)
                        continue
                    nc.sync.dma_start(
                        sub_ap(ring, slot * LC * S, [[1, rows * S]]),
                        dram_ap(xb + c * LC * S, rows),
                    )
                    compute(c, slot, out_sb)

                # store on the Activation HWDGE queue (never blocks loads)
                nc.scalar.dma_start(
                    bass.AP(y.ap().tensor, b * TP, [[LS, 128], [1, LS]]),
                    out_sb[:],
                )

    nc.compile()
    return nc


def _get_nc():
    if "nc" not in _cache:
        _cache["nc"] = _build()
    return _cache["nc"]


def kernel(microphone_array: np.ndarray) -> np.ndarray:
    import ml_dtypes
    from concourse.bass_utils import run_bass_kernel_spmd

    bf16 = np.dtype(ml_dtypes.bfloat16)
    x = np.asarray(microphone_array, dtype=np.float32)
    assert x.shape == (B, T, S)
    nc = _get_nc()

    scale = np.float32(1.0 / S)  # power of two: exact under bf16 rounding
    in_maps = []
    for c in range(NCORES):
        shard = np.zeros((BC, TPAD, S), dtype=bf16)
        shard[:, :T] = (x[c * BC : (c + 1) * BC] * scale).astype(bf16)
        in_maps.append({"x": shard.reshape(-1)})

    res = _cache["res"] = run_bass_kernel_spmd(
        nc, in_maps, core_ids=list(range(NCORES)), trace=_cache.get("trace", False)
    )

    out = np.empty((B, T), dtype=np.float32)
    for c in range(NCORES):
        out[c * BC : (c + 1) * BC] = (
            res.results[c]["y"].reshape(BC, TP)[:, :T].astype(np.float32)
        )
    return out


# revision 5
# speedup vs baseline: 1.7017x; 1.2583x over previous
"""Delay-and-sum beamformer on 8 TRN2 NeuronCores (bf16 pipeline).

Problem: x[16, 100000, 128] f32 -> out[b, t] = mean_s x[b, t + d_s, s],
d_s = round(s * sin(30deg) / 2) in [0, 32] (zero-padded past t = T-1).

Sharding: pure data parallel over batch (2 batches per core).

The rel-err gate is 2e-2; bf16 round-to-nearest costs ~4e-3, so the host
shard step converts to bf16 (folding the 1/128 mean scale into the
conversion -- a power of two, so it is exact) and the device pipeline is
bf16 end to end.  This halves DMA bytes and enables the DVE 2x perf mode
(2-byte dtype, packed inner dim) for the big strided reduce.

Per-core layout ("stripe" scheme): for each batch, partition p owns time
rows [784*p, 784*(p+1)) of a zero-padded T_pad = 100384 signal, rows
contiguous in the SBUF free dim as [row, sensor] (row pitch 128 bf16 =
256 B).  Rows stream through a 6-slot ring of 98-row chunks (8 chunks
per batch) in DESCENDING time order, so a chunk's +32-row halo is the
already-resident next chunk at the adjacent slot.  The ring has one
32-row shadow after slot 5; chunks landing on slot 5 are loaded "fat"
(130 rows) so their compute window is self-contained.  With 8 chunks
over 6 slots the first/last chunks of a batch never collide on a slot,
so batch handoff does not stall the queue.

The delayed sensor sum exploits the delay structure: d groups sensors as
{0,1,2} (d=0), {4k-1..4k+2} (d=k, k=1..31), {127} (d=32).  Per chunk:
  DVE:    g1[tau] = strided reduce of the 31 4-wide groups (2x mode)
  GPSIMD: g2[tau] = reduce of sensors {0,1,2} at row tau,
          g2 += x[tau+32, 127]
  DVE:    out[tau] = g1 + g2   (input was pre-scaled by 1/128)
Loads run on the SP HWDGE queue; output stores run on the Activation
HWDGE queue so they never head-of-line-block the loads.
"""

import numpy as np

B, T, S = 16, 100000, 128
NCORES = 8
BC = B // NCORES          # batches per core
LS = 784                  # stripe rows per partition (128*784 = 100352 >= T)
TP = 128 * LS             # padded output rows per batch
HALO = 32                 # max delay
TPAD = TP + HALO          # padded input rows per batch
LC = 98                   # chunk rows
NCH = LS // LC            # 8 chunks per batch
NSLOT = 6                 # ring slots
OFF = 4                   # slot(c) = (c + OFF) % NSLOT; c=7 -> 5 (shadowed)
RING_ROWS = NSLOT * LC + HALO   # 620 rows * 256 B = 158.7 KiB / partition

_cache = {}


def _build():
    import concourse.bass as bass
    import concourse.tile as tile
    from concourse import bacc, mybir

    bf16 = mybir.dt.bfloat16
    nc = bacc.Bacc("TRN2", target_bir_lowering=False, debug=False, num_devices=1)
    x = nc.dram_tensor("x", [BC * TPAD * S], bf16, kind="ExternalInput")
    y = nc.dram_tensor("y", [BC * TP], bf16, kind="ExternalOutput")

    def dram_ap(base_elem, rows):
        # [128 partitions (stripe-major), rows*S contiguous elems each]
        return bass.AP(x.ap().tensor, base_elem, [[LS * S, 128], [1, rows * S]])

    def sub_ap(t, off, dims):
        # custom AP into a tile: keep its partition dim, replace free dims
        return bass.AP(t.tensor, t.offset + off, [list(t.ap[0])] + dims)

    with tile.TileContext(nc) as tc:
        from contextlib import ExitStack

        with ExitStack() as ctx:
            ctx.enter_context(
                nc.allow_low_precision(
                    reason="bf16 sums; rel-err gate is 2e-2, bf16 costs ~4e-3"
                )
            )
            ring_pool = ctx.enter_context(tc.tile_pool(name="ring", bufs=1))
            f_pool = ctx.enter_context(tc.tile_pool(name="f", bufs=1))
            p2_pool = ctx.enter_context(tc.tile_pool(name="p2", bufs=2))
            o_pool = ctx.enter_context(tc.tile_pool(name="o", bufs=2))

            ring = ring_pool.tile([128, RING_ROWS * S], bf16)
            p1t = f_pool.tile([128, LC * 62], bf16)
            v16t = f_pool.tile([128, LC * 16], bf16)
            v8t = f_pool.tile([128, LC * 8], bf16)
            v4t = f_pool.tile([128, LC * 4], bf16)
            v2t = f_pool.tile([128, LC * 2], bf16)

            def compute(c, slot, out_sb, t0=0, t1=LC):
                # produce out rows [c*LC+t0, c*LC+t1) from ring rows
                # [slot*LC+t0, slot*LC+t1+32).  InstReduce never gets the DVE
                # bf16 2x mode but InstTensorTensor does, so the 128-tap sum
                # is a binary fold tree of tensor_add ops.
                n = t1 - t0
                base = slot * LC * S + t0 * S
                p2t = p2_pool.tile([128, n * 32], bf16, tag="p2")
                # L1: pair delay groups k=0..14 with k=15..29 (inner [1,4])
                nc.vector.tensor_add(
                    sub_ap(p1t, 0, [[62, n], [4, 15], [1, 4]]),
                    sub_ap(ring, base + 131, [[S, n], [132, 15], [1, 4]]),
                    sub_ap(ring, base + 131 + 132 * 15, [[S, n], [132, 15], [1, 4]]),
                )
                # L1b: straggler group k=30 folded to 2 values
                nc.vector.tensor_add(
                    sub_ap(p1t, 60, [[62, n], [1, 2]]),
                    sub_ap(ring, base + 131 + 132 * 30, [[S, n], [1, 2]]),
                    sub_ap(ring, base + 133 + 132 * 30, [[S, n], [1, 2]]),
                )
                # g2 = sensors {0,1,2} at row tau + sensor 127 at row tau+32,
                # on GPSIMD, deposited as p2 column 31 so F16 folds it in.
                g2 = sub_ap(p2t, 31, [[32, n]])
                nc.gpsimd.tensor_add(
                    g2,
                    sub_ap(ring, base, [[S, n]]),
                    sub_ap(ring, base + 1, [[S, n]]),
                )
                nc.gpsimd.tensor_add(g2, g2, sub_ap(ring, base + 2, [[S, n]]))
                nc.gpsimd.tensor_add(
                    g2, g2, sub_ap(ring, base + HALO * S + 127, [[S, n]])
                )
                # L2: 62 -> 31 (packed [1,31])
                nc.vector.tensor_add(
                    sub_ap(p2t, 0, [[32, n], [1, 31]]),
                    sub_ap(p1t, 0, [[62, n], [1, 31]]),
                    sub_ap(p1t, 31, [[62, n], [1, 31]]),
                )
                # F16/F8/F4/F2: packed halvings (g2 rides along at col 31)
                nc.vector.tensor_add(
                    sub_ap(v16t, 0, [[16, n], [1, 16]]),
                    sub_ap(p2t, 0, [[32, n], [1, 16]]),
                    sub_ap(p2t, 16, [[32, n], [1, 16]]),
                )
                nc.vector.tensor_add(
                    sub_ap(v8t, 0, [[8, n], [1, 8]]),
                    sub_ap(v16t, 0, [[16, n], [1, 8]]),
                    sub_ap(v16t, 8, [[16, n], [1, 8]]),
                )
                nc.vector.tensor_add(
                    sub_ap(v4t, 0, [[4, n], [1, 4]]),
                    sub_ap(v8t, 0, [[8, n], [1, 4]]),
                    sub_ap(v8t, 4, [[8, n], [1, 4]]),
                )
                nc.vector.tensor_add(
                    sub_ap(v2t, 0, [[2, n], [1, 2]]),
                    sub_ap(v4t, 0, [[4, n], [1, 2]]),
                    sub_ap(v4t, 2, [[4, n], [1, 2]]),
                )
                # F1 -> output slice
                nc.vector.tensor_add(
                    out_sb[:, c * LC + t0 : c * LC + t1],
                    sub_ap(v2t, 0, [[2, n]]),
                    sub_ap(v2t, 1, [[2, n]]),
                )

            for b in range(BC):
                xb = b * TPAD * S
                out_sb = o_pool.tile([128, LS], bf16, tag="out_sb")
                for c in range(NCH - 1, -1, -1):
                    slot = (c + OFF) % NSLOT
                    rows = LC + HALO if slot == NSLOT - 1 else LC
                    if b == BC - 1 and c == 0:
                        # last chunk of the kernel: split load+compute to
                        # shorten the end-of-kernel drain.
                        mid = LC // 2
                        nc.sync.dma_start(
                            sub_ap(ring, (slot * LC + mid) * S, [[1, (LC - mid) * S]]),
                            dram_ap(xb + (c * LC + mid) * S, LC - mid),
                        )
                        compute(c, slot, out_sb, mid, LC)
                        nc.sync.dma_start(
                            sub_ap(ring, slot * LC * S, [[1, mid * S]]),
                            dram_ap(xb + c * LC * S, mid),
                        )
                        compute(c, slot, out_sb, 0, mid)
                        continue
                    nc.sync.dma_start(
                        sub_ap(ring, slot * LC * S, [[1, rows * S]]),
                        dram_ap(xb + c * LC * S, rows),
                    )
                    compute(c, slot, out_sb)

                # store on the Activation HWDGE queue (never blocks loads)
                nc.scalar.dma_start(
                    bass.AP(y.ap().tensor, b * TP, [[LS, 128], [1, LS]]),
                    out_sb[:],
                )

    nc.compile()
    return nc


def _get_nc():
    if "nc" not in _cache:
        _cache["nc"] = _build()
    return _cache["nc"]


def kernel(microphone_array: np.ndarray) -> np.ndarray:
    import ml_dtypes
    from concourse.bass_utils import run_bass_kernel_spmd

    bf16 = np.dtype(ml_dtypes.bfloat16)
    x = np.asarray(microphone_array, dtype=np.float32)
    assert x.shape == (B, T, S)
    nc = _get_nc()

    scale = np.float32(1.0 / S)  # power of two: exact under bf16 rounding
    in_maps = []
    for c in range(NCORES):
        shard = np.zeros((BC, TPAD, S), dtype=bf16)
        shard[:, :T] = (x[c * BC : (c + 1) * BC] * scale).astype(bf16)
        in_maps.append({"x": shard.reshape(-1)})

    res = _cache["res"] = run_bass_kernel_spmd(
        nc, in_maps, core_ids=list(range(NCORES)), trace=_cache.get("trace", False)
    )

    out = np.empty((B, T), dtype=np.float32)
    for c in range(NCORES):
        out[c * BC : (c + 1) * BC] = (
            res.results[c]["y"].reshape(BC, TP)[:, :T].astype(np.float32)
        )
    return out
